# revision 1
# baseline (speedup 1.0000x reference)
"""Cosine-similarity kernel (x[16384,512] vs weights[4096,512] -> [16384,4096])
on 8 Trainium2 NeuronCores, data-parallel over the x batch dim.

Per core: x shard [2048,512] fp32, full weights [4096,512] fp32.
  out = normalize(x) @ normalize(w).T
Implemented as: raw x -> PE transpose -> f32r; w -> row-normalize -> PE
transpose -> f32r; f32r matmuls (1 cyc/row) accumulate K=512 in PSUM;
x-row 1/norm applied as ACT scale at PSUM eviction.
"""
import numpy as np

B, D, N = 16384, 512, 4096
NCORES = 8
BS = B // NCORES          # 2048 rows per core
MT = BS // 128            # 16 m-tiles
NT = N // 128             # 32 w row-tiles
KC = D // 128             # 4 k-chunks
NB = N // 512             # 8 n-blocks of 512

_cached = {}


def _build():
    import concourse.bass as bass
    import concourse.mybir as mybir
    import concourse.tile as tile
    from concourse import bacc
    from concourse.masks import make_identity

    F32, F32R = mybir.dt.float32, mybir.dt.float32r
    nc = bacc.Bacc(None, target_bir_lowering=False)
    x = nc.dram_tensor("x", [BS, D], F32, kind="ExternalInput")
    w = nc.dram_tensor("weights", [N, D], F32, kind="ExternalInput")
    o = nc.dram_tensor("out", [BS, N], F32, kind="ExternalOutput")

    with tile.TileContext(nc) as tc:
        with (
            tc.tile_pool(name="const", bufs=1) as const,
            tc.tile_pool(name="big", bufs=1) as big,
            tc.tile_pool(name="stage", bufs=6) as stage,
            tc.tile_pool(name="ostage", bufs=8) as ostage,
            tc.tile_pool(name="mmps", bufs=4, space="PSUM") as mmps,
            tc.tile_pool(name="trps", bufs=4, space="PSUM") as trps,
        ):
            ident = const.tile([128, 128], F32, name="ident")
            make_identity(nc, ident[:])
            rx = const.tile([128, MT], F32, name="rx")

            wT = [big.tile([128, N], F32R, name=f"wT{k}") for k in range(KC)]
            xT = [big.tile([128, BS], F32R, name=f"xT{k}") for k in range(KC)]

            def w_prep(j):
                wt = stage.tile([128, D], F32, name="wt", tag="wt")
                nc.sync.dma_start(wt[:], w[j * 128:(j + 1) * 128, :])
                sq = stage.tile([128, D], F32, name="sq", tag="sq")
                ss = stage.tile([128, 1], F32, name="ss", tag="ss")
                nc.scalar.activation(
                    sq[:], wt[:], mybir.ActivationFunctionType.Square,
                    accum_out=ss[:])
                inv = stage.tile([128, 1], F32, name="inv", tag="inv")
                nc.vector.reciprocal(inv[:], ss[:])
                rw = stage.tile([128, 1], F32, name="rw", tag="rw")
                nc.scalar.sqrt(rw[:], inv[:])
                wn = stage.tile([128, D], F32, name="wn", tag="wn")
                nc.scalar.mul(wn[:], wt[:], rw[:])
                for k in range(KC):
                    pt = trps.tile([128, 128], F32, name="pt", tag="pt")
                    nc.tensor.transpose(pt[:], wn[:, k * 128:(k + 1) * 128], ident[:])
                    nc.vector.tensor_copy(wT[k][:, j * 128:(j + 1) * 128], pt[:])

            # ---- x: load, compute 1/norm, transpose raw to [K, BS] f32r ----
            for m in range(MT):
                xt = stage.tile([128, D], F32, name="xt", tag="wt")
                nc.sync.dma_start(xt[:], x[m * 128:(m + 1) * 128, :])
                sq = stage.tile([128, D], F32, name="sqx", tag="sq")
                ss = stage.tile([128, 1], F32, name="ssx", tag="ss")
                nc.scalar.activation(
                    sq[:], xt[:], mybir.ActivationFunctionType.Square,
                    accum_out=ss[:])
                inv = stage.tile([128, 1], F32, name="invx", tag="inv")
                nc.vector.reciprocal(inv[:], ss[:])
                nc.scalar.sqrt(rx[:, m:m + 1], inv[:])
                for k in range(KC):
                    pt = trps.tile([128, 128], F32, name="ptx", tag="pt")
                    nc.tensor.transpose(pt[:], xt[:, k * 128:(k + 1) * 128], ident[:])
                    nc.vector.tensor_copy(xT[k][:, m * 128:(m + 1) * 128], pt[:])

            # ---- main GEMM, pipelined with w-prep per n-block column ----
            for nb in range(NB):
                for j in range(nb * 4, nb * 4 + 4):
                    w_prep(j)
                for m in range(MT):
                    pm = mmps.tile([128, 512], F32, name="pm", tag="pm")
                    for k in range(KC):
                        nc.tensor.matmul(
                            pm[:],
                            xT[k][:, m * 128:(m + 1) * 128],
                            wT[k][:, nb * 512:(nb + 1) * 512],
                            start=(k == 0), stop=(k == KC - 1))
                    ot = ostage.tile([128, 512], F32, name="ot", tag="ot")
                    nc.scalar.mul(ot[:], pm[:], rx[:, m:m + 1])
                    nc.sync.dma_start(
                        o[m * 128:(m + 1) * 128, nb * 512:(nb + 1) * 512], ot[:])
    nc.compile()
    return nc


def kernel(x: np.ndarray, weights: np.ndarray) -> np.ndarray:
    from concourse.bass_utils import run_bass_kernel_spmd

    if "nc" not in _cached:
        _cached["nc"] = _build()
    nc = _cached["nc"]

    x = np.ascontiguousarray(x, dtype=np.float32)
    weights = np.ascontiguousarray(weights, dtype=np.float32)
    in_maps = [
        {"x": x[i * BS:(i + 1) * BS], "weights": weights} for i in range(NCORES)
    ]
    res = run_bass_kernel_spmd(nc, in_maps, list(range(NCORES)))
    return np.concatenate([res.results[i]["out"] for i in range(NCORES)], axis=0)



# revision 2
# speedup vs baseline: 1.9051x; 1.9051x over previous
"""Cosine-similarity kernel (x[16384,512] vs weights[4096,512] -> [16384,4096])
on 8 Trainium2 NeuronCores, data-parallel over the x batch dim.

Strategy: cos(x, w) = (xn/|xn|)·(wn/|wn|) is a normalized GEMM.  All cheap
O(B*D) prep runs on host: normalize, scale by S, quantize to TRN fp8 e4m3,
and pre-pack transposed k-tile-paired layouts.  The device does only the
O(B*N*D) GEMM as DoubleRow fp8 matmuls (2 k-rows per partition), PSUM
accumulation, fp16 eviction, and big contiguous DMAs.

Precision: w-side is sent as a two-term fp8 expansion (w8 + s8), so the
device computes x8·(w8+s8) [+ optionally r8·w8 over half of K], which keeps
max|err|/absmax(ref) under the 2e-2 gate (measured 1.90e-2 / 1.38e-2 on the
fixed problem seed).
"""
import numpy as np
import ml_dtypes

B, D, N = 16384, 512, 4096
NCORES = 8
BS = B // NCORES          # 2048 rows per core
MT = BS // 128            # 16 m-tiles
SCALE = 128.0             # fp8 dynamic-range scale; out = psum / SCALE^2
NCHUNK = 4                # n-column chunks of 1024 for DMA/compute overlap
R_COMP = 1                # 1: add r8·w8 over K/2 (rel~1.38e-2), 0: rel~1.90e-2

E4 = ml_dtypes.float8_e4m3  # IEEE-style e4m3, max normal 240 == TRN FP8_EXP4

_cached = {}


def _build():
    import concourse.bass as bass
    import concourse.mybir as mybir
    import concourse.tile as tile
    from concourse import bacc

    F32, F16, F8 = mybir.dt.float32, mybir.dt.float16, mybir.dt.float8e4
    DR = mybir.MatmulPerfMode.DoubleRow

    nc = bacc.Bacc(None, target_bir_lowering=False)
    x8d = [nc.dram_tensor(f"x8_{g}", [128, 2, BS], F8, kind="ExternalInput")
           for g in range(2)]
    w8d = [nc.dram_tensor(f"w8_{g}", [128, 2, N], F8, kind="ExternalInput")
           for g in range(2)]
    s8d = [nc.dram_tensor(f"s8_{g}", [128, 2, N], F8, kind="ExternalInput")
           for g in range(2)]
    r8d = (nc.dram_tensor("r8", [128, 2, BS], F8, kind="ExternalInput")
           if R_COMP else None)
    outd = nc.dram_tensor("out", [BS, N], F16, kind="ExternalOutput")

    with tile.TileContext(nc) as tc:
        with (
            tc.tile_pool(name="ops", bufs=1) as ops,
            tc.tile_pool(name="ostage", bufs=4) as ostage,
            tc.tile_pool(name="mmps", bufs=4, space="PSUM") as mmps,
        ):
            x8t = [ops.tile([128, 2, BS], F8, name=f"x8t{g}") for g in range(2)]
            w8t = [ops.tile([128, 2, N], F8, name=f"w8t{g}") for g in range(2)]
            s8t = [ops.tile([128, 2, N], F8, name=f"s8t{g}") for g in range(2)]
            r8t = ops.tile([128, 2, BS], F8, name="r8t") if R_COMP else None

            # x-side first (needed by every block), then w/s per n-chunk so
            # the first chunk's matmuls start after ~6us instead of ~16us.
            for g in range(2):
                nc.sync.dma_start(x8t[g][:], x8d[g][:])
            if R_COMP:
                nc.sync.dma_start(r8t[:], r8d[:])
            for c in range(NCHUNK):
                cs = slice(c * (N // NCHUNK), (c + 1) * (N // NCHUNK))
                for g in range(2):
                    nc.sync.dma_start(w8t[g][:, :, cs], w8d[g][:, :, cs])
                    nc.sync.dma_start(s8t[g][:, :, cs], s8d[g][:, :, cs])

            ev = 0
            for c in range(NCHUNK):
                for m in range(MT):
                    ms = slice(m * 128, (m + 1) * 128)
                    ot = ostage.tile([128, N // NCHUNK], F16, name="ot", tag="ot")
                    for nbh in range(2):
                        nb = c * 2 + nbh
                        ns = slice(nb * 512, (nb + 1) * 512)
                        pm = mmps.tile([128, 512], F32, name="pm", tag="pm")
                        nc.tensor.matmul(pm[:], x8t[0][:, :, ms], w8t[0][:, :, ns],
                                         start=True, stop=False, perf_mode=DR)
                        nc.tensor.matmul(pm[:], x8t[1][:, :, ms], w8t[1][:, :, ns],
                                         start=False, stop=False, perf_mode=DR)
                        nc.tensor.matmul(pm[:], x8t[0][:, :, ms], s8t[0][:, :, ns],
                                         start=False, stop=False, perf_mode=DR)
                        nc.tensor.matmul(pm[:], x8t[1][:, :, ms], s8t[1][:, :, ns],
                                         start=False, stop=(not R_COMP),
                                         perf_mode=DR)
                        if R_COMP:
                            nc.tensor.matmul(pm[:], r8t[:, :, ms],
                                             w8t[0][:, :, ns],
                                             start=False, stop=True, perf_mode=DR)
                        dst = ot[:, nbh * 512:(nbh + 1) * 512]
                        if ev % 2 == 0:
                            nc.scalar.copy(dst, pm[:])
                        else:
                            nc.vector.tensor_copy(dst, pm[:])
                        ev += 1
                    nc.sync.dma_start(
                        outd[ms, c * (N // NCHUNK):(c + 1) * (N // NCHUNK)],
                        ot[:])
    nc.compile()
    return nc


def _q8(a):
    return np.clip(a, -240.0, 240.0).astype(E4)


def _pack(t8):
    """[rows, 512] fp8 -> per k-group g: [128 (d'), 2 (slot j), rows] with
    d = g*256 + j*128 + d', matching the DoubleRow operand layout."""
    a = np.ascontiguousarray(t8.T).reshape(2, 2, 128, t8.shape[0])
    a = a.transpose(0, 2, 1, 3)
    return [np.ascontiguousarray(a[g]) for g in range(2)]


def kernel(x: np.ndarray, weights: np.ndarray) -> np.ndarray:
    from concourse.bass_utils import run_bass_kernel_spmd

    if "nc" not in _cached:
        _cached["nc"] = _build()
    nc = _cached["nc"]

    x = np.ascontiguousarray(x, dtype=np.float32)
    w = np.ascontiguousarray(weights, dtype=np.float32)
    xn = x * (SCALE / np.maximum(np.linalg.norm(x, axis=1, keepdims=True), 1e-8))
    wn = w * (SCALE / np.maximum(np.linalg.norm(w, axis=1, keepdims=True), 1e-8))

    x8 = _q8(xn)
    w8 = _q8(wn)
    s8 = _q8(wn - w8.astype(np.float32))
    xp = _pack(x8)
    wp = _pack(w8)
    sp = _pack(s8)
    if R_COMP:
        r8 = _q8(xn - x8.astype(np.float32))
        rp = _pack(r8)

    in_maps = []
    for i in range(NCORES):
        bs = slice(i * BS, (i + 1) * BS)
        m = {
            "x8_0": np.ascontiguousarray(xp[0][:, :, bs]),
            "x8_1": np.ascontiguousarray(xp[1][:, :, bs]),
            "w8_0": wp[0], "w8_1": wp[1],
            "s8_0": sp[0], "s8_1": sp[1],
        }
        if R_COMP:
            m["r8"] = np.ascontiguousarray(rp[0][:, :, bs])
        in_maps.append(m)

    res = run_bass_kernel_spmd(nc, in_maps, list(range(NCORES)))
    out = np.concatenate([res.results[i]["out"] for i in range(NCORES)], axis=0)
    return out.astype(np.float32) * np.float32(1.0 / (SCALE * SCALE))


# revision 5
# speedup vs baseline: 2.4678x; 1.2954x over previous
"""Cosine-similarity kernel (x[16384,512] vs weights[4096,512] -> [16384,4096])
on 8 Trainium2 NeuronCores, data-parallel over the x batch dim.

Strategy: cos(x, w) = (xn/|xn|)·(wn/|wn|) is a normalized GEMM.  All cheap
O(B*D) prep runs on host: normalize, scale by S, quantize to TRN fp8 e4m3,
and pre-pack transposed k-tile-paired layouts.  The device does only the
O(B*N*D) GEMM as DoubleRow fp8 matmuls (2 k-rows per partition), PSUM
accumulation, fp16 eviction, and big contiguous DMAs.

Precision: w-side is sent as a two-term fp8 expansion (w8 + s8), so the
device computes x8·(w8+s8) [+ optionally r8·w8 over half of K], which keeps
max|err|/absmax(ref) under the 2e-2 gate (measured 1.90e-2 / 1.38e-2 on the
fixed problem seed).
"""
import numpy as np
import ml_dtypes

B, D, N = 16384, 512, 4096
NCORES = 8
BS = B // NCORES          # 2048 rows per core
MT = BS // 128            # 16 m-tiles
SCALE = 128.0             # fp8 dynamic-range scale; out = psum / SCALE^2
NCHUNK = 4                # n-column chunks of 1024 for DMA/compute overlap
R_COMP = 0                # 1: add r8·w8 over K/2 (rel~1.38e-2), 0: rel~1.90e-2

E4 = ml_dtypes.float8_e4m3  # IEEE-style e4m3, max normal 240 == TRN FP8_EXP4

_cached = {}


def _build():
    import concourse.bass as bass
    import concourse.mybir as mybir
    import concourse.tile as tile
    from concourse import bacc

    F32, F16, F8 = mybir.dt.float32, mybir.dt.float16, mybir.dt.float8e4
    DR = mybir.MatmulPerfMode.DoubleRow

    nc = bacc.Bacc(None, target_bir_lowering=False)
    x8d = [nc.dram_tensor(f"x8_{g}", [128, 2, BS], F8, kind="ExternalInput")
           for g in range(2)]
    w8d = [nc.dram_tensor(f"w8_{g}", [128, 2, N], F8, kind="ExternalInput")
           for g in range(2)]
    s8d = [nc.dram_tensor(f"s8_{g}", [128, 2, N], F8, kind="ExternalInput")
           for g in range(2)]
    r8d = (nc.dram_tensor("r8", [128, 2, BS], F8, kind="ExternalInput")
           if R_COMP else None)
    outd = nc.dram_tensor("out", [BS, N], F16, kind="ExternalOutput")

    with tile.TileContext(nc) as tc:
        with (
            tc.tile_pool(name="ops", bufs=1) as ops,
            tc.tile_pool(name="ostage", bufs=10) as ostage,
            tc.tile_pool(name="mmps", bufs=6, space="PSUM") as mmps,
        ):
            x8t = [ops.tile([128, 2, BS], F8, name=f"x8t{g}") for g in range(2)]
            w8t = [ops.tile([128, 2, N], F8, name=f"w8t{g}") for g in range(2)]
            s8t = [ops.tile([128, 2, N], F8, name=f"s8t{g}") for g in range(2)]
            r8t = ops.tile([128, 2, BS], F8, name="r8t") if R_COMP else None

            # x-side first (needed by every block), then w/s per n-chunk so
            # the first chunk's matmuls start after ~6us instead of ~16us.
            for g in range(2):
                nc.sync.dma_start(x8t[g][:], x8d[g][:])
            for c in range(NCHUNK):
                cs = slice(c * (N // NCHUNK), (c + 1) * (N // NCHUNK))
                for g in range(2):
                    nc.sync.dma_start(w8t[g][:, :, cs], w8d[g][:, :, cs])
                    nc.sync.dma_start(s8t[g][:, :, cs], s8d[g][:, :, cs])
                if c == 0 and R_COMP:
                    nc.sync.dma_start(r8t[:], r8d[:])

            ev = 0
            for c in range(NCHUNK):
                for m in range(MT):
                    ms = slice(m * 128, (m + 1) * 128)
                    ot = ostage.tile([128, N // NCHUNK], F16, name="ot", tag="ot")
                    for nbh in range(2):
                        nb = c * 2 + nbh
                        ns = slice(nb * 512, (nb + 1) * 512)
                        pm = mmps.tile([128, 512], F32, name="pm", tag="pm")
                        nc.tensor.matmul(pm[:], x8t[0][:, :, ms], w8t[0][:, :, ns],
                                         start=True, stop=False, perf_mode=DR)
                        nc.tensor.matmul(pm[:], x8t[1][:, :, ms], w8t[1][:, :, ns],
                                         start=False, stop=False, perf_mode=DR)
                        nc.tensor.matmul(pm[:], x8t[0][:, :, ms], s8t[0][:, :, ns],
                                         start=False, stop=False, perf_mode=DR)
                        nc.tensor.matmul(pm[:], x8t[1][:, :, ms], s8t[1][:, :, ns],
                                         start=False, stop=(not R_COMP),
                                         perf_mode=DR)
                        if R_COMP:
                            nc.tensor.matmul(pm[:], r8t[:, :, ms],
                                             w8t[0][:, :, ns],
                                             start=False, stop=True, perf_mode=DR)
                        dst = ot[:, nbh * 512:(nbh + 1) * 512]
                        if ev % 2 == 0:
                            nc.scalar.copy(dst, pm[:])
                        else:
                            nc.vector.tensor_copy(dst, pm[:])
                        ev += 1
                    nc.sync.dma_start(
                        outd[ms, c * (N // NCHUNK):(c + 1) * (N // NCHUNK)],
                        ot[:])
    nc.compile()
    return nc


def _q8(a):
    return np.clip(a, -240.0, 240.0).astype(E4)


def _pack(t8):
    """[rows, 512] fp8 -> per k-group g: [128 (d'), 2 (slot j), rows] with
    d = g*256 + j*128 + d', matching the DoubleRow operand layout."""
    a = np.ascontiguousarray(t8.T).reshape(2, 2, 128, t8.shape[0])
    a = a.transpose(0, 2, 1, 3)
    return [np.ascontiguousarray(a[g]) for g in range(2)]


def kernel(x: np.ndarray, weights: np.ndarray) -> np.ndarray:
    from concourse.bass_utils import run_bass_kernel_spmd

    if "nc" not in _cached:
        _cached["nc"] = _build()
    nc = _cached["nc"]

    x = np.ascontiguousarray(x, dtype=np.float32)
    w = np.ascontiguousarray(weights, dtype=np.float32)
    xn = x * (SCALE / np.maximum(np.linalg.norm(x, axis=1, keepdims=True), 1e-8))
    wn = w * (SCALE / np.maximum(np.linalg.norm(w, axis=1, keepdims=True), 1e-8))

    x8 = _q8(xn)
    w8 = _q8(wn)
    s8 = _q8(wn - w8.astype(np.float32))
    xp = _pack(x8)
    wp = _pack(w8)
    sp = _pack(s8)
    if R_COMP:
        r8 = _q8(xn - x8.astype(np.float32))
        rp = _pack(r8)

    in_maps = []
    for i in range(NCORES):
        bs = slice(i * BS, (i + 1) * BS)
        m = {
            "x8_0": np.ascontiguousarray(xp[0][:, :, bs]),
            "x8_1": np.ascontiguousarray(xp[1][:, :, bs]),
            "w8_0": wp[0], "w8_1": wp[1],
            "s8_0": sp[0], "s8_1": sp[1],
        }
        if R_COMP:
            m["r8"] = np.ascontiguousarray(rp[0][:, :, bs])
        in_maps.append(m)

    res = run_bass_kernel_spmd(nc, in_maps, list(range(NCORES)))
    out = np.concatenate([res.results[i]["out"] for i in range(NCORES)], axis=0)
    return out.astype(np.float32) * np.float32(1.0 / (SCALE * SCALE))
